# revision 2
# baseline (speedup 1.0000x reference)
"""Trainium2 Bass kernel for the 2-layer per-timestep-weight custom RNN.

Strategy: data-parallel over batch across 8 NeuronCores (weights replicated).
Each core processes B/8=16 batch rows through both layers sequentially in t.
On-chip layout is transposed ([H partitions, batch free]) so the per-timestep
weight matrices stream straight from HBM into SBUF as matmul stationaries and
activations get per-partition biases.

The 0.5 * (n1 + n2) output scaling is folded into the weights on the host
(state s = 2*h is kept on-device; Wh[:,0] and Win1 are pre-scaled by 0.5,
outputs post-scaled by 0.5 on the host), which removes one vector op per cell.
"""

import sys

for _p in ("/opt/trn_rl_repo",):
    if _p not in sys.path:
        sys.path.insert(0, _p)

import numpy as np

import concourse.bass as bass  # noqa: F401
import concourse.tile as tile
from concourse import bacc, mybir
from concourse.bass_utils import run_bass_kernel_spmd

B, T, D, H = 128, 64, 256, 256
NCORES = 8
BC = B // NCORES  # batch rows per core
CH = 4  # timesteps per weight-stream DMA chunk
NCHUNK = T // CH

USE_BF16 = True

if USE_BF16:
    WDT = mybir.dt.bfloat16
    import ml_dtypes

    NPDT = ml_dtypes.bfloat16
    WBUFS = 3
else:
    WDT = mybir.dt.float32
    NPDT = np.float32
    WBUFS = 2

F32 = mybir.dt.float32
AF = mybir.ActivationFunctionType
ALU = mybir.AluOpType


def _build_nc():
    nc = bacc.Bacc("TRN2", target_bir_lowering=False, debug=False, num_devices=NCORES)

    # Per-core inputs (x/state shards differ per core; weights replicated).
    xt_d = nc.dram_tensor("xt", [128, 2, T, BC], WDT, kind="ExternalInput")
    sin0_d = nc.dram_tensor("sin0", [128, 32], WDT, kind="ExternalInput")
    sin1_d = nc.dram_tensor("sin1", [128, 32], WDT, kind="ExternalInput")
    w_d = {
        (0, "win"): nc.dram_tensor("w0", [NCHUNK, 128, CH, 2, H], WDT, kind="ExternalInput"),
        (0, "wh"): nc.dram_tensor("wh0", [NCHUNK, 128, CH, 3, 2, H], WDT, kind="ExternalInput"),
        (1, "win"): nc.dram_tensor("w1", [NCHUNK, 128, CH, 2, H], WDT, kind="ExternalInput"),
        (1, "wh"): nc.dram_tensor("wh1", [NCHUNK, 128, CH, 3, 2, H], WDT, kind="ExternalInput"),
    }
    b_d = [
        nc.dram_tensor("bias0", [128, T, 3, 2], F32, kind="ExternalInput"),
        nc.dram_tensor("bias1", [128, T, 3, 2], F32, kind="ExternalInput"),
    ]
    out_d = nc.dram_tensor("out", [128, T, 32], WDT, kind="ExternalOutput")
    hfin_d = nc.dram_tensor("hfin", [2, 128, 32], WDT, kind="ExternalOutput")

    with tile.TileContext(nc) as tc:
        with (
            tc.tile_pool(name="persist", bufs=1) as persist,
            tc.tile_pool(name="weights", bufs=WBUFS) as wpool,
            tc.tile_pool(name="acts", bufs=3) as acts,
            tc.tile_pool(name="psum", bufs=1, space="PSUM") as psum,
        ):
            # ---- static loads -------------------------------------------
            xt_sb = persist.tile([128, 2, T, BC], WDT, tag="xt")
            nc.sync.dma_start(xt_sb[:], xt_d[:])
            bias_sb = []
            for l in range(2):
                bt = persist.tile([128, T, 3, 2], F32, tag=f"bias{l}")
                nc.sync.dma_start(bt[:], b_d[l][:])
                bias_sb.append(bt)
            out_sb = persist.tile([128, T, 32], WDT, tag="out")

            s0_prev = acts.tile([128, 32], WDT, tag="s0")
            nc.sync.dma_start(s0_prev[:], sin0_d[:])
            sin1_sb = persist.tile([128, 32], WDT, tag="sin1")
            nc.sync.dma_start(sin1_sb[:], sin1_d[:])

            # ---- weight chunk streaming ---------------------------------
            chunk_tiles = {}

            def issue_chunk(c):
                tiles = {}
                for l in range(2):
                    wi = wpool.tile([128, CH, 2, H], WDT, tag=f"win{l}")
                    nc.sync.dma_start(wi[:], w_d[(l, "win")][c])
                    wh = wpool.tile([128, CH, 3, 2, H], WDT, tag=f"wh{l}")
                    nc.sync.dma_start(wh[:], w_d[(l, "wh")][c])
                    tiles[l] = (wi, wh)
                chunk_tiles[c] = tiles

            for c in range(min(WBUFS - 1, NCHUNK)):
                issue_chunk(c)

            # ---- one RNN cell, as a generator of pipeline stages --------
            def cell_stages(l, t, in_sl, state_sl, wi, wh, ti, s_out_ap):
                bs = bias_sb[l]
                p0 = psum.tile([128, 32], F32, tag=f"p0_{l}")
                for m in range(2):
                    o = p0[:, m * 16:(m + 1) * 16]
                    ms = slice(m * 128, (m + 1) * 128)
                    nc.tensor.matmul(o, wi[:, ti, 0, ms], in_sl[0], start=True, stop=False)
                    nc.tensor.matmul(o, wi[:, ti, 1, ms], in_sl[1], start=False, stop=False)
                    nc.tensor.matmul(o, wh[:, ti, 0, 0, ms], state_sl[0], start=False, stop=False)
                    nc.tensor.matmul(o, wh[:, ti, 0, 1, ms], state_sl[1], start=False, stop=True)
                yield
                n0 = acts.tile([128, 32], WDT, tag=f"n0_{l}")
                for m in range(2):
                    sl = slice(m * 16, (m + 1) * 16)
                    nc.scalar.activation(n0[:, sl], p0[:, sl], AF.Tanh, bias=bs[:, t, 0, m:m+1])
                yield
                p1 = psum.tile([128, 32], F32, tag=f"p1_{l}")
                for m in range(2):
                    o = p1[:, m * 16:(m + 1) * 16]
                    ms = slice(m * 128, (m + 1) * 128)
                    nc.tensor.matmul(o, wh[:, ti, 1, 0, ms], n0[:, 0:16], start=True, stop=False)
                    nc.tensor.matmul(o, wh[:, ti, 1, 1, ms], n0[:, 16:32], start=False, stop=True)
                yield
                r1 = acts.tile([128, 32], WDT, tag=f"r1_{l}")
                for m in range(2):
                    sl = slice(m * 16, (m + 1) * 16)
                    nc.vector.tensor_scalar(r1[:, sl], p1[:, sl], bs[:, t, 1, m:m+1], 0.0, ALU.add, ALU.max)
                n1 = acts.tile([128, 32], WDT, tag=f"n1_{l}")
                nc.vector.tensor_add(n1[:], r1[:], n0[:])
                yield
                p2 = psum.tile([128, 32], F32, tag=f"p2_{l}")
                for m in range(2):
                    o = p2[:, m * 16:(m + 1) * 16]
                    ms = slice(m * 128, (m + 1) * 128)
                    nc.tensor.matmul(o, wh[:, ti, 2, 0, ms], n1[:, 0:16], start=True, stop=False)
                    nc.tensor.matmul(o, wh[:, ti, 2, 1, ms], n1[:, 16:32], start=False, stop=True)
                yield
                a = acts.tile([128, 32], WDT, tag=f"a_{l}")
                nc.vector.tensor_add(a[:], n1[:], n0[:])
                sg = acts.tile([128, 32], WDT, tag=f"sg_{l}")
                for m in range(2):
                    sl = slice(m * 16, (m + 1) * 16)
                    nc.scalar.activation(sg[:, sl], p2[:, sl], AF.Sigmoid, bias=bs[:, t, 2, m:m+1])
                yield
                nc.vector.tensor_add(s_out_ap, a[:], sg[:])

            def drive(g0, g1):
                done0 = done1 = False
                while not (done0 and done1):
                    if not done0:
                        done0 = next(g0, "END") == "END"
                    if not done1:
                        done1 = next(g1, "END") == "END"

            empty = iter(())

            # ---- main time loop (layer-1 pipelined one step behind) -----
            prev_g1 = empty
            s1_state = [sin1_sb[:, 0:16], sin1_sb[:, 16:32]]
            for t in range(T):
                c, ti = divmod(t, CH)
                if ti == 0 and c + (WBUFS - 1) < NCHUNK:
                    issue_chunk(c + (WBUFS - 1))
                wi0, wh0 = chunk_tiles[c][0]
                s0_new = acts.tile([128, 32], WDT, tag="s0")
                g0 = cell_stages(
                    0, t,
                    [xt_sb[:, 0, t, :], xt_sb[:, 1, t, :]],
                    [s0_prev[:, 0:16], s0_prev[:, 16:32]],
                    wi0, wh0, ti, s0_new[:],
                )
                drive(g0, prev_g1)
                wi1, wh1 = chunk_tiles[c][1]
                prev_g1 = cell_stages(
                    1, t,
                    [s0_new[:, 0:16], s0_new[:, 16:32]],
                    s1_state,
                    wi1, wh1, ti, out_sb[:, t, :],
                )
                s1_state = [out_sb[:, t, 0:16], out_sb[:, t, 16:32]]
                s0_prev = s0_new
            drive(empty, prev_g1)

            # ---- results back to DRAM -----------------------------------
            nc.sync.dma_start(hfin_d[0], s0_prev[:])
            nc.sync.dma_start(hfin_d[1], out_sb[:, T - 1, :])
            nc.sync.dma_start(out_d[:], out_sb[:])

    nc.compile()
    return nc


_NC_CACHE = {}


def _get_nc():
    if "nc" not in _NC_CACHE:
        _NC_CACHE["nc"] = _build_nc()
    return _NC_CACHE["nc"]


def _prep_inputs(x, hidden, Win0, Wh0, b0, Win1, Wh1, b1):
    """Host-side fold + relayout. Returns per-core input maps."""
    x = np.asarray(x, np.float32)
    hidden = np.asarray(hidden, np.float32)
    Wh0f = np.array(Wh0, np.float32)
    Wh0f[:, 0] *= 0.5
    Wh1f = np.array(Wh1, np.float32)
    Wh1f[:, 0] *= 0.5
    Win1f = np.asarray(Win1, np.float32) * 0.5
    Win0f = np.asarray(Win0, np.float32)

    # weights → [chunk, p, ti, (n,) k2, h]
    w0 = Win0f.reshape(NCHUNK, CH, 2, 128, H).transpose(0, 3, 1, 2, 4)
    w1 = Win1f.reshape(NCHUNK, CH, 2, 128, H).transpose(0, 3, 1, 2, 4)
    wh0 = Wh0f.reshape(NCHUNK, CH, 3, 2, 128, H).transpose(0, 4, 1, 2, 3, 5)
    wh1 = Wh1f.reshape(NCHUNK, CH, 3, 2, 128, H).transpose(0, 4, 1, 2, 3, 5)
    w0 = np.ascontiguousarray(w0, NPDT)
    w1 = np.ascontiguousarray(w1, NPDT)
    wh0 = np.ascontiguousarray(wh0, NPDT)
    wh1 = np.ascontiguousarray(wh1, NPDT)

    bias0 = np.ascontiguousarray(
        np.asarray(b0, np.float32).reshape(T, 3, 2, 128).transpose(3, 0, 1, 2)
    )
    bias1 = np.ascontiguousarray(
        np.asarray(b1, np.float32).reshape(T, 3, 2, 128).transpose(3, 0, 1, 2)
    )

    # x → per-core [p, k2, t, b]
    xt = x.reshape(NCORES, BC, T, 2, 128).transpose(0, 4, 3, 2, 1)
    xt = np.ascontiguousarray(xt, NPDT)
    # initial state s = 2h → per-core [l, p, (h2 b)]
    s = (2.0 * hidden).reshape(2, NCORES, BC, 2, 128).transpose(1, 0, 4, 3, 2)
    s = np.ascontiguousarray(s.reshape(NCORES, 2, 128, 32), NPDT)

    in_maps = []
    for c in range(NCORES):
        in_maps.append({
            "xt": xt[c], "sin0": s[c, 0], "sin1": s[c, 1],
            "w0": w0, "wh0": wh0, "w1": w1, "wh1": wh1,
            "bias0": bias0, "bias1": bias1,
        })
    return in_maps


def _run(in_maps, trace=False):
    nc = _get_nc()
    return run_bass_kernel_spmd(nc, in_maps, list(range(NCORES)), trace=trace)


def _postprocess(results):
    # out: [p, t, (h2 b)] per core → output [B, T, H]; hfin → hidden [2, B, H]
    output = np.empty((B, T, H), np.float32)
    hidden_out = np.empty((2, B, H), np.float32)
    for c in range(NCORES):
        o = np.asarray(results[c]["out"], np.float32).reshape(128, T, 2, BC)
        # o[p, t, h2, b] = s1; h = 0.5*s
        output[c * BC:(c + 1) * BC] = 0.5 * o.transpose(3, 1, 2, 0).reshape(BC, T, H)
        hf = np.asarray(results[c]["hfin"], np.float32).reshape(2, 128, 2, BC)
        hidden_out[:, c * BC:(c + 1) * BC, :] = 0.5 * hf.transpose(0, 3, 2, 1).reshape(2, BC, H)
    return output, hidden_out


def kernel(x, hidden, Win0, Wh0, b0, Win1, Wh1, b1):
    in_maps = _prep_inputs(x, hidden, Win0, Wh0, b0, Win1, Wh1, b1)
    res = _run(in_maps, trace=False)
    return _postprocess(res.results)
